# revision 16
# baseline (speedup 1.0000x reference)
"""TRN2 Bass kernel for the E2E DRO module.

Math (per scenario t, vmapped over 2048 scenarios):
  Y_hat = X @ W.T + b;  ep = Y - Y_hat  (shared)
  64 projected-subgradient steps on (z in simplex, c free, lam >= 0) of the
  TV-DRO objective; returns (Z_star, Y_hat).

Gradient reformulation used on device (validated vs jax.grad):
  s = ep@z - c; u = s^2; umax = amax^2 (amax = max|s|); i* = argmax|s|
  m_i = |s_i| > t1,  t1 = sqrt(relu(umax - 2*lam));  cnt = sum(m); beta = 1 - cnt/n
  gz = (2/n) ep^T (m.s) + 2 beta s* ep[i*] - y_hat
  gc = -(2/n) 1^T (m.s)  - 2 beta s*
  glam = rho - 2 + 2 cnt / n
With epaug = [ep | -1] (n x 65) and z_aug = [z; c], the matmul terms come from
epaug^T @ (m~ . s~) where s~ = s / t1' (t1' = max(t1, 1e-20)) is computed by a
second matmul with rescaled z_aug so the mask threshold is the constant 1.0.
cnt comes from a bf16 ones^T @ mask matmul (masks are exact in bf16).

Sharding: data-parallel over scenarios, 256 per core; ep/W/b replicated.
"""

import os
import numpy as np
from contextlib import ExitStack

import concourse.bass as bass
import concourse.tile as tile
from concourse import bacc, mybir
from concourse.bass_utils import run_bass_kernel_spmd
from concourse.masks import make_identity

F32 = mybir.dt.float32
BF16 = mybir.dt.bfloat16
U32 = mybir.dt.uint32
AL = mybir.AluOpType
AF = mybir.ActivationFunctionType
AX = mybir.AxisListType.X

N, NX, NY, E, P = 2048, 128, 64, 65, 128
NCH = N // P            # 16 obs chunks
NCORES = 8
TCORE = N // NCORES     # 256 scenarios per core
NT = TCORE // P         # 2 scenario tiles per core
N_ITER, LR0 = int(os.environ.get('NITER', '64')), 0.05
DBG = bool(int(os.environ.get('DBG', '0')))
DBGT = int(os.environ.get('DBGT', '0'))
K_PROJ = 5              # Newton/Michelot rounds
TINY = 1e-20

# mask-mult groups routed to the Pool engine (of 8 groups)
POOLG = set(int(x) for x in os.environ.get("POOLG", "").split(",") if x != "")
# projection count-pass engine: 1 -> Pool, 0 -> DVE
POOLCNT = bool(int(os.environ.get("POOLCNT", "0")))
MTDT_ENV = os.environ.get("MTDT", "bf16")

_PROG_CACHE = {}
MTDT = None  # set in _build


def _f32(x):
    return float(np.float32(x))


def _build(rho_val: float):
    global MTDT
    MTDT = BF16 if MTDT_ENV == "bf16" else F32
    nc = bacc.Bacc("TRN2", target_bir_lowering=False, debug=False,
                   num_devices=NCORES)
    X_d = nc.dram_tensor("X", [N, NX], F32, kind="ExternalInput")
    Y_d = nc.dram_tensor("Y", [N, NY], F32, kind="ExternalInput")
    Xs_d = nc.dram_tensor("Xs", [TCORE, NX], F32, kind="ExternalInput")
    W_d = nc.dram_tensor("W", [NY, NX], F32, kind="ExternalInput")
    b_d = nc.dram_tensor("b", [NY], F32, kind="ExternalInput")
    Z_d = nc.dram_tensor("Z", [TCORE, NY], F32, kind="ExternalOutput")
    YH_d = nc.dram_tensor("YH", [TCORE, NY], F32, kind="ExternalOutput")
    epaug_d = nc.dram_tensor("epaug_scratch", [N, E], F32)  # for row gathers
    if DBG:
        dbg_amax = nc.dram_tensor("dbg_amax", [P, NT], F32, kind="ExternalOutput")
        dbg_cnt = nc.dram_tensor("dbg_cnt", [P, NT], F32, kind="ExternalOutput")
        dbg_t1p = nc.dram_tensor("dbg_t1p", [P, NT], F32, kind="ExternalOutput")
        dbg_sstar = nc.dram_tensor("dbg_sstar", [P, NT], F32, kind="ExternalOutput")
        dbg_V0 = nc.dram_tensor("dbg_V0", [P, E], F32, kind="ExternalOutput")
        dbg_RT0 = nc.dram_tensor("dbg_RT0", [P, E], F32, kind="ExternalOutput")
        dbg_EPG0 = nc.dram_tensor("dbg_EPG0", [P, E], F32, kind="ExternalOutput")

    with tile.TileContext(nc) as tc, ExitStack() as ctx:
        gsem = ctx.enter_context(nc.semaphore("gather_sem"))
        gsem_val = [0]
        const = ctx.enter_context(tc.tile_pool(name="const", bufs=1))
        state = ctx.enter_context(tc.tile_pool(name="state", bufs=1))
        big = ctx.enter_context(tc.tile_pool(name="big", bufs=1))
        work = ctx.enter_context(tc.tile_pool(name="work", bufs=2))
        tiny = ctx.enter_context(tc.tile_pool(name="tiny", bufs=6))
        ps_big = ctx.enter_context(tc.tile_pool(name="ps_big", bufs=4, space="PSUM"))
        ps_sm = ctx.enter_context(tc.tile_pool(name="ps_sm", bufs=2, space="PSUM"))
        ps_acc = ctx.enter_context(tc.tile_pool(name="ps_acc", bufs=1, space="PSUM"))

        # ---------------- constants / precompute ----------------
        ident = const.tile([P, P], F32, tag="ident")
        make_identity(nc, ident[:])
        ones1 = const.tile([1, P], F32, tag="ones1")
        nc.vector.memset(ones1[:], 1.0)
        onescol_f = const.tile([P, 1], F32, tag="onescolf")
        nc.vector.memset(onescol_f[:], 1.0)
        onescol_bf = const.tile([P, 1], BF16, tag="onescolbf")
        nc.vector.memset(onescol_bf[:], 1.0)

        # W -> Wt (128x x 64y)
        w_sb = tiny.tile([NY, NX], F32, tag="wsb")
        nc.gpsimd.dma_start(w_sb[:], W_d.ap())
        ps_w = ps_sm.tile([NX, NY], F32, tag="pss")
        nc.tensor.transpose(out=ps_w[:], in_=w_sb[:], identity=ident[0:NY, 0:NY])
        Wt = const.tile([NX, NY], F32, tag="Wt")
        nc.scalar.copy(Wt[:], ps_w[:])
        b_row = const.tile([1, NY], F32, tag="brow")
        nc.gpsimd.dma_start(b_row[:], b_d.ap().rearrange("(o d) -> o d", o=1))

        # X -> XT (128x x 2048obs)
        XT = big.tile([NX, N], F32, tag="XT")
        for k in range(NCH):
            xst = work.tile([P, NX], F32, tag="xstage")
            nc.gpsimd.dma_start(xst[:], X_d.ap()[k * P:(k + 1) * P, :])
            ps_x = ps_sm.tile([NX, P], F32, tag="pss")
            nc.tensor.transpose(out=ps_x[:], in_=xst[:], identity=ident[:])
            nc.scalar.copy(XT[:, k * P:(k + 1) * P], ps_x[:])
        # Xs -> XsT (128x x 256scen)
        XsT = const.tile([NX, TCORE], F32, tag="XsT")
        for m in range(NT):
            xst = work.tile([P, NX], F32, tag="xstage")
            nc.gpsimd.dma_start(xst[:], Xs_d.ap()[m * P:(m + 1) * P, :])
            ps_x = ps_sm.tile([NX, P], F32, tag="pss")
            nc.tensor.transpose(out=ps_x[:], in_=xst[:], identity=ident[:])
            nc.scalar.copy(XsT[:, m * P:(m + 1) * P], ps_x[:])

        # Y chunks (128 x 16*64)
        Ysb = big.tile([P, NCH, NY], F32, tag="Ysb")
        nc.gpsimd.dma_start(
            Ysb[:], Y_d.ap().rearrange("(k p) d -> p k d", p=P))

        # EPA (128 x 16 x 65) obs-layout epaug; epaugT (65 x 2048)
        EPA = big.tile([P, NCH, E], F32, tag="EPA")
        epaugT = big.tile([E, N], F32, tag="epaugT")
        for k in range(NCH):
            ps_yh = ps_sm.tile([P, NY], F32, tag="pss")
            nc.tensor.matmul(out=ps_yh[:], lhsT=XT[:, k * P:(k + 1) * P],
                             rhs=Wt[:], start=True, stop=False)
            nc.tensor.matmul(out=ps_yh[:], lhsT=ones1[:], rhs=b_row[:],
                             start=False, stop=True)
            nc.vector.tensor_tensor(out=EPA[:, k, 0:NY], in0=Ysb[:, k, :],
                                    in1=ps_yh[:], op=AL.subtract)
        nc.vector.memset(EPA[:, :, NY:E], -1.0)
        for k in range(NCH):
            ps_t = ps_sm.tile([E, P], F32, tag="pss")
            nc.tensor.transpose(out=ps_t[:], in_=EPA[:, k, :], identity=ident[:])
            nc.scalar.copy(epaugT[:, k * P:(k + 1) * P], ps_t[:])
        # epaug to DRAM for row gathers
        nc.gpsimd.dma_start(
            epaug_d.ap().rearrange("(k p) e -> p k e", p=P), EPA[:])

        # Y_hat slice (scen layout) + output + YHaug state
        YHaug = [state.tile([P, E], F32, tag=f"YHaug{m}", name=f"YHaug{m}")
                 for m in range(NT)]
        for m in range(NT):
            ps_yh = ps_sm.tile([P, NY], F32, tag="pss")
            nc.tensor.matmul(out=ps_yh[:], lhsT=XsT[:, m * P:(m + 1) * P],
                             rhs=Wt[:], start=True, stop=False)
            nc.tensor.matmul(out=ps_yh[:], lhsT=ones1[:], rhs=b_row[:],
                             start=False, stop=True)
            nc.scalar.copy(YHaug[m][:, 0:NY], ps_yh[:])
            nc.vector.memset(YHaug[m][:, NY:E], 0.0)
            nc.gpsimd.dma_start(YH_d.ap()[m * P:(m + 1) * P, :],
                                YHaug[m][:, 0:NY])

        # c0 = mean(ep @ z0) over all obs, z0 uniform
        rowsums = tiny.tile([NY, 1], F32, tag="rows")
        nc.vector.tensor_reduce(out=rowsums[:], in_=epaugT[0:NY, :], axis=AX,
                                op=AL.add)
        ps_c0 = ps_sm.tile([1, 1], F32, tag="pss")
        nc.tensor.matmul(out=ps_c0[:], lhsT=rowsums[:], rhs=onescol_f[0:NY, :],
                         start=True, stop=True)
        c0s = tiny.tile([1, 1], F32, tag="c0s")
        nc.scalar.copy(c0s[:], ps_c0[:])
        ps_c0b = ps_sm.tile([P, 1], F32, tag="pss")
        nc.tensor.matmul(out=ps_c0b[:], lhsT=ones1[:], rhs=c0s[:],
                         start=True, stop=True)
        c0col = tiny.tile([P, 1], F32, tag="c0col")
        nc.vector.tensor_scalar(out=c0col[:], in0=ps_c0b[:],
                                scalar1=_f32(1.0 / (N * NY)), scalar2=None,
                                op0=AL.mult)

        # states (lam2 packed [P, NT])
        ZaT = [state.tile([P, E], F32, tag=f"ZaT{m}", name=f"ZaT{m}")
               for m in range(NT)]
        lam2 = state.tile([P, NT], F32, tag="lam2", name="lam2")
        for m in range(NT):
            nc.vector.memset(ZaT[m][:, 0:NY], _f32(1.0 / NY))
            nc.vector.tensor_copy(ZaT[m][:, NY:E], c0col[:])
        nc.vector.memset(lam2[:], 2.0)

        # persistent per-iteration tiles
        A_sb = [state.tile([P, N], F32, tag=f"Asb{m}", name=f"Asb{m}")
                for m in range(NT)]
        EPG = [[state.tile([P, E], F32, tag=f"EPG{m}_{j}", name=f"EPG{m}_{j}")
                for j in range(2)] for m in range(NT)]

        # ---------------- iterations ----------------
        for t in range(N_ITER):
            lr = _f32(np.float32(LR0) / np.float32(np.sqrt(np.float32(t + 1.0))))
            jbuf = t % 2
            Za_e = work.tile([E, TCORE], F32, tag="Za_e")
            Zt_e = work.tile([E, TCORE], F32, tag="Zt_e")

            # -- stage A: Za transposes, S matmuls, |S| chunks, amax --
            for m in range(NT):
                ps_z = ps_sm.tile([E, P], F32, tag="pss")
                nc.tensor.transpose(out=ps_z[:], in_=ZaT[m][:], identity=ident[:])
                nc.scalar.copy(Za_e[:, m * P:(m + 1) * P], ps_z[:])
            amx2 = tiny.tile([P, NT, 4], F32, tag="amx2", name="amx2")
            for m in range(NT):
                for h in range(4):
                    ps_S = ps_big.tile([P, 512], F32, tag="psbig",
                                       name=f"psS{m}{h}")
                    nc.tensor.matmul(
                        out=ps_S[:],
                        lhsT=Za_e[:, m * P:(m + 1) * P],
                        rhs=epaugT[:, h * 512:(h + 1) * 512],
                        start=True, stop=True)
                    nc.scalar.activation(A_sb[m][:, h * 512:(h + 1) * 512],
                                         ps_S[:], AF.Abs)
                    nc.vector.tensor_reduce(
                        out=amx2[:, m, h:h + 1],
                        in_=A_sb[m][:, h * 512:(h + 1) * 512],
                        axis=AX, op=AL.max)

            # Vb = ZaT + lr*YHaug (early; off critical path)
            Vb = [None] * NT
            for m in range(NT):
                Vb[m] = work.tile([P, E], F32, tag=f"Vb{m}", name=f"Vb{m}")
                nc.vector.scalar_tensor_tensor(
                    out=Vb[m][:], in0=YHaug[m][:], scalar=_f32(lr),
                    in1=ZaT[m][:], op0=AL.mult, op1=AL.add)

            amax = tiny.tile([P, NT], F32, tag="amax", name="amax")
            nc.vector.tensor_reduce(out=amax[:], in_=amx2[:], axis=AX,
                                    op=AL.max)

            # -- stage B: packed scalar chain -> rt1, t1p; Zt rescale --
            umax = tiny.tile([P, NT], F32, tag="umax")
            nc.scalar.activation(umax[:], amax[:], AF.Square)
            t1sq = tiny.tile([P, NT], F32, tag="t1sq")
            nc.vector.tensor_tensor(out=t1sq[:], in0=umax[:], in1=lam2[:],
                                    op=AL.subtract)
            nc.vector.tensor_scalar(out=t1sq[:], in0=t1sq[:], scalar1=1e-30,
                                    scalar2=None, op0=AL.max)
            t1p = tiny.tile([P, NT], F32, tag="t1p", name="t1p")
            nc.scalar.activation(t1p[:], t1sq[:], AF.Sqrt)
            rt1 = tiny.tile([P, NT], F32, tag="rt1", name="rt1")
            nc.vector.reciprocal(rt1[:], t1p[:])
            for m in range(NT):
                ZtT = tiny.tile([P, E], F32, tag=f"ZtT{m}")
                nc.vector.tensor_scalar(out=ZtT[:], in0=ZaT[m][:],
                                        scalar1=rt1[:, m:m + 1], scalar2=None,
                                        op0=AL.mult)
                ps_zt = ps_sm.tile([E, P], F32, tag="pss")
                nc.tensor.transpose(out=ps_zt[:], in_=ZtT[:], identity=ident[:])
                nc.scalar.copy(Zt_e[:, m * P:(m + 1) * P], ps_zt[:])

            # -- argmax / gather (overlaps stage B/C) --
            sstar = tiny.tile([P, NT], F32, tag="sstar", name="sstar")
            for m in range(NT):
                amax8 = tiny.tile([P, 8], F32, tag=f"amax8{m}")
                nc.vector.tensor_copy(amax8[:],
                                      amax[:, m:m + 1].to_broadcast([P, 8]))
                idx8 = tiny.tile([P, 8], U32, tag=f"idx8{m}")
                nc.vector.max_index(idx8[:], amax8[:], A_sb[m][:])
                with tc.tile_critical(name="gather"):
                    gsem_val[0] += 16
                    nc.gpsimd.indirect_dma_start(
                        out=EPG[m][jbuf][:], out_offset=None, in_=epaug_d.ap(),
                        in_offset=bass.IndirectOffsetOnAxis(
                            ap=idx8[:, 0:1], axis=0)).then_inc(gsem, 16)
                    nc.gpsimd.wait_ge(gsem, gsem_val[0])

            # -- stage C: obs-layout rescaled S~, mask, masked values,
            #    R and cnt accumulation matmuls (pipelined per group) --
            Mt = work.tile([P, NCH * TCORE], MTDT, tag="Mt")
            Wm = work.tile([P, NCH * TCORE], F32, tag="Wm")
            ps_R = ps_acc.tile([E, TCORE], F32, tag="psR", name="psR")
            ps_cnt = ps_acc.tile([1, TCORE], F32, tag="pscnt", name="pscnt")
            ag_t = [None] * 8
            ps_g = [None] * 8

            def emit_st(g):
                ps_st = ps_big.tile([P, 2 * TCORE], F32, tag="psbig",
                                    name=f"psst{g}")
                for k2 in range(2):
                    k = 2 * g + k2
                    nc.tensor.matmul(
                        out=ps_st[:, k2 * TCORE:(k2 + 1) * TCORE],
                        lhsT=epaugT[:, k * P:(k + 1) * P],
                        rhs=Zt_e[:], start=True, stop=True)
                ps_g[g] = ps_st

            def emit_mask(g):
                sl = slice(g * 2 * TCORE, (g + 1) * 2 * TCORE)
                ag = work.tile([P, 2 * TCORE], F32, tag="ag", name=f"ag{g % 2}")
                nc.scalar.activation(ag[:], ps_g[g][:], AF.Abs)
                nc.vector.tensor_scalar(out=Mt[:, sl], in0=ag[:],
                                        scalar1=1.0, scalar2=None,
                                        op0=AL.is_gt)
                eng = nc.gpsimd if g in POOLG else nc.vector
                eng.tensor_tensor(out=Wm[:, sl], in0=Mt[:, sl],
                                  in1=ps_g[g][:], op=AL.mult)
                ag_t[g] = ag

            def emit_contract(g):
                for k2 in range(2):
                    k = 2 * g + k2
                    sl = slice(k * TCORE, (k + 1) * TCORE)
                    nc.tensor.matmul(out=ps_R[:],
                                     lhsT=EPA[:, k, :],
                                     rhs=Wm[:, sl],
                                     start=(k == 0), stop=(k == NCH - 1))
                    nc.tensor.matmul(out=ps_cnt[:],
                                     lhsT=(onescol_bf[:] if MTDT is BF16 else onescol_f[:]),
                                     rhs=Mt[:, sl],
                                     start=(k == 0), stop=(k == NCH - 1))

            # software-pipelined emission: st(g) runs ahead of contract(g-2)
            emit_st(0)
            emit_mask(0)
            emit_st(1)
            emit_mask(1)
            for g in range(2, 8):
                emit_st(g)
                emit_contract(g - 2)
                emit_mask(g)
            emit_contract(6)
            emit_contract(7)

            # sstar = dot(EPG, Za) per m (after gather lands)
            for m in range(NT):
                dotscr = tiny.tile([P, E], F32, tag=f"dotscr{m}",
                                   name=f"dotscr{m}")
                nc.vector.scalar_tensor_tensor(
                    out=dotscr[:], in0=EPG[m][jbuf][:], scalar=1.0,
                    in1=ZaT[m][:], op0=AL.mult, op1=AL.mult,
                    accum_out=sstar[:, m:m + 1])

            # -- stage D: R/cnt to scen layout, coefficient chains --
            R_sb = work.tile([E, TCORE], F32, tag="Rsb")
            nc.scalar.copy(R_sb[:], ps_R[:])
            cnt_row = tiny.tile([1, TCORE], F32, tag="cntrow")
            nc.scalar.copy(cnt_row[:], ps_cnt[:])
            # cnt transposes first (freed fast by the tiny copies), then RT
            # transposes which stay live until the V-update stt reads them.
            ps_cT = [None] * NT
            for m in range(NT):
                ps_cT[m] = ps_sm.tile([P, 1], F32, tag="pss", name=f"pscT{m}")
                nc.tensor.transpose(out=ps_cT[m][:],
                                    in_=cnt_row[:, m * P:(m + 1) * P],
                                    identity=ident[0:1, 0:1])
            cnt2 = tiny.tile([P, NT], F32, tag="cnt2", name="cnt2")
            for m in range(NT):
                nc.vector.tensor_copy(cnt2[:, m:m + 1], ps_cT[m][:])
            ps_RT = [None] * NT
            for m in range(NT):
                ps_RT[m] = ps_sm.tile([P, E], F32, tag="pss", name=f"psRT{m}")
                nc.tensor.transpose(out=ps_RT[m][:],
                                    in_=R_sb[:, m * P:(m + 1) * P],
                                    identity=ident[0:E, 0:E])

            # beta = 1 - cnt/N ; nk2 = -2*lr*beta*sstar ; nk1 = -2*lr/N*t1p
            betaf = tiny.tile([P, NT], F32, tag="betaf")
            nc.vector.tensor_scalar(out=betaf[:], in0=cnt2[:],
                                    scalar1=_f32(-1.0 / N), scalar2=1.0,
                                    op0=AL.mult, op1=AL.add)
            nk2 = tiny.tile([P, NT], F32, tag="nk2")
            nc.vector.tensor_tensor(out=nk2[:], in0=betaf[:], in1=sstar[:],
                                    op=AL.mult)
            nc.vector.tensor_scalar(out=nk2[:], in0=nk2[:],
                                    scalar1=_f32(-2.0 * lr), scalar2=None,
                                    op0=AL.mult)
            nk1 = tiny.tile([P, NT], F32, tag="nk1")
            nc.vector.tensor_scalar(out=nk1[:], in0=t1p[:],
                                    scalar1=_f32(-lr * 2.0 / N), scalar2=None,
                                    op0=AL.mult)
            # lam2 = relu(lam2 - (4lr/N)cnt - 2lr(rho-2))   [x2 of lam step]
            g1 = tiny.tile([P, NT], F32, tag="g1")
            nc.vector.tensor_scalar(out=g1[:], in0=cnt2[:],
                                    scalar1=_f32(4.0 * lr / N),
                                    scalar2=_f32(2.0 * lr * (rho_val - 2.0)),
                                    op0=AL.mult, op1=AL.add)
            nc.vector.tensor_tensor(out=lam2[:], in0=lam2[:], in1=g1[:],
                                    op=AL.subtract)
            nc.vector.tensor_scalar(out=lam2[:], in0=lam2[:],
                                    scalar1=0.0, scalar2=None, op0=AL.max)

            # V = Vb + nk1*RT + nk2*EPG ; projection
            V = [None] * NT
            sv0 = tiny.tile([P, NT], F32, tag="sv0")
            for m in range(NT):
                V[m] = work.tile([P, E], F32, tag=f"V{m}", name=f"V{m}")
                nc.vector.scalar_tensor_tensor(
                    out=V[m][:], in0=ps_RT[m][:], scalar=nk1[:, m:m + 1],
                    in1=Vb[m][:], op0=AL.mult, op1=AL.add)
                nc.vector.scalar_tensor_tensor(
                    out=V[m][:], in0=EPG[m][jbuf][:], scalar=nk2[:, m:m + 1],
                    in1=V[m][:], op0=AL.mult, op1=AL.add)
                nc.vector.tensor_reduce(out=sv0[:, m:m + 1], in_=V[m][:, 0:NY],
                                        axis=AX, op=AL.add)
            if DBG and t == DBGT:
                nc.gpsimd.dma_start(dbg_amax.ap(), amax[:])
                nc.gpsimd.dma_start(dbg_cnt.ap(), cnt2[:])
                nc.gpsimd.dma_start(dbg_t1p.ap(), t1p[:])
                nc.gpsimd.dma_start(dbg_sstar.ap(), sstar[:])
                nc.gpsimd.dma_start(dbg_V0.ap(), V[0][:])
                scr = work.tile([P, E], F32, tag="dbgscr", name="dbgscr")
                nc.vector.tensor_copy(scr[:], ps_RT[0][:])
                nc.gpsimd.dma_start(dbg_RT0.ap(), scr[:])
                nc.gpsimd.dma_start(dbg_EPG0.ap(), EPG[0][jbuf][:])

            # ---- simplex projection (Newton on theta, K rounds) ----
            thn = [None] * NT
            nthn = [None] * NT
            for m in range(NT):
                thn[m] = tiny.tile([P, 1], F32, tag=f"thn{m}", name=f"thn{m}")
                nc.vector.tensor_scalar(out=thn[m][:], in0=sv0[:, m:m + 1],
                                        scalar1=1.0, scalar2=_f32(-1.0 / NY),
                                        op0=AL.subtract, op1=AL.mult)
                nthn[m] = tiny.tile([P, 1], F32, tag=f"nthn{m}", name=f"nthn{m}")
                nc.vector.tensor_scalar(out=nthn[m][:], in0=sv0[:, m:m + 1],
                                        scalar1=1.0, scalar2=_f32(1.0 / NY),
                                        op0=AL.subtract, op1=AL.mult)
            for r in range(K_PROJ):
                for m in range(NT):
                    sv = tiny.tile([P, 1], F32, tag=f"sv{m}")
                    pscr = tiny.tile([P, NY], F32, tag=f"pscr{m}")
                    nc.scalar.activation(pscr[:], V[m][:, 0:NY], AF.Relu,
                                         bias=thn[m][:, 0:1], scale=1.0,
                                         accum_out=sv[:])
                    cmf = tiny.tile([P, 1], F32, tag=f"cm{m}")
                    pscr2 = tiny.tile([P, NY], F32, tag=f"pscr2{m}")
                    eng = nc.gpsimd if POOLCNT else nc.vector
                    eng.tensor_scalar(out=pscr2[:], in0=V[m][:, 0:NY],
                                      scalar1=nthn[m][:, 0:1], scalar2=None,
                                      op0=AL.is_gt, op1=AL.add,
                                      accum_out=cmf[:])
                    rc = tiny.tile([P, 1], F32, tag=f"rc{m}")
                    nc.vector.reciprocal(rc[:], cmf[:])
                    d = tiny.tile([P, 1], F32, tag=f"d{m}")
                    nc.vector.scalar_tensor_tensor(
                        out=d[:], in0=sv[:], scalar=1.0, in1=rc[:],
                        op0=AL.subtract, op1=AL.mult)
                    nc.vector.tensor_tensor(out=thn[m][:], in0=thn[m][:],
                                            in1=d[:], op=AL.subtract)
                    if r < K_PROJ - 1:
                        nc.vector.tensor_tensor(out=nthn[m][:], in0=nthn[m][:],
                                                in1=d[:], op=AL.add)
            for m in range(NT):
                nc.scalar.activation(ZaT[m][:, 0:NY], V[m][:, 0:NY], AF.Relu,
                                     bias=thn[m][:, 0:1], scale=1.0)
                nc.vector.tensor_copy(ZaT[m][:, NY:E], V[m][:, NY:E])

        # ---------------- output ----------------
        for m in range(NT):
            nc.gpsimd.dma_start(Z_d.ap()[m * P:(m + 1) * P, :],
                                ZaT[m][:, 0:NY])

    nc.compile()
    return nc


def kernel(X, Y, rho, W, b):
    X = np.ascontiguousarray(X, np.float32)
    Y = np.ascontiguousarray(Y, np.float32)
    W = np.ascontiguousarray(W, np.float32)
    b = np.ascontiguousarray(b, np.float32)
    rho_val = float(np.float32(rho[0]))
    key = rho_val
    if key not in _PROG_CACHE:
        _PROG_CACHE[key] = _build(rho_val)
    nc = _PROG_CACHE[key]
    in_maps = [{
        "X": X, "Y": Y, "W": W, "b": b,
        "Xs": np.ascontiguousarray(X[c * TCORE:(c + 1) * TCORE]),
    } for c in range(NCORES)]
    res = run_bass_kernel_spmd(nc, in_maps, list(range(NCORES))).results
    Z = np.concatenate([res[c]["Z"] for c in range(NCORES)], 0)
    YH = np.concatenate([res[c]["YH"] for c in range(NCORES)], 0)
    return Z, YH
